# revision 18
# baseline (speedup 1.0000x reference)
"""Trainium2 Bass kernel for GQA attention with RoPE (dense_transformer).

Reference computation (per batch b):
    q = x @ wq  -> [T, 32, 64],  k = x @ wk -> [T, 8, 64], v = x @ wv
    rope(q), rope(k); scores = q k^T / 8; w = softmax(scores); out = (w v) @ wo

Sharding over 8 NeuronCores: 2 batch groups x 4-way head tensor parallel.
Core c: batch b=c//4, head group g=c%4 (q-heads 8g..8g+8, kv-heads 2g,2g+1).
Within a group of 4 cores the attention outputs (transposed, [512,T]) are
AllGather'd per 512-column t-chunk; each core then computes a 512-column
slice of out = attn @ wo.

v2 schedule (single pipeline, engine-balanced):
  - Phase A: projections + RoPE with HAM-warmup matmuls and DMA-chased
    accumulation.  Q/K/V all projected up front; qt/ktd/vaug persist.
  - Phase B: per 512-col t-chunk, per head-pair: software-pipelined s-loop
    emitting QK(s+1) -> exp(s) -> PV(s).  QK pairs are row-tiled (K=64 on
    partitions 0:64/64:128) so both heads' score matmuls run concurrently.
  - exp is column-split across engines per tile: ACT handles cols
    [0:SPLITC], DVE handles [SPLITC:1024] via a Schraudolph bf16-bits
    tensor_scalar (int16(x*128/ln2 + const) == bf16 bits of e^x, ~1.5%
    elementwise, cancels via the softmax denominator common mode).
  - The softmax denominator is the 65th (ones) column of the V stationary,
    so it falls out of the PV matmul for free; normalization happens at
    PSUM->SBUF evacuation (DVE muls by a gpsimd-broadcast reciprocal).
  - wo matmul groups for chunk c-1 are emitted between chunk c's pairs:
    they fill the PE while ACT/DVE/gpsimd run the den/normalize tail, and
    the AllGather latency of chunk c-1 hides under chunk c's attention.
"""

import numpy as np
import ml_dtypes

import concourse.bass as bass
import concourse.mybir as mybir
import concourse.tile as tile
from concourse import bacc
from concourse.bass_utils import run_bass_kernel_spmd

BF16 = mybir.dt.bfloat16
F32 = mybir.dt.float32
I16 = mybir.dt.int16

T = 2048          # sequence length (also s dim)
C = 2048          # model dim
HD = 64           # head dim
DQ = 512          # q dims per core (8 heads)
DKV = 128         # kv dims per core (2 kv heads)
N_CORES = 8
THETA = 10000.0

EXP = mybir.ActivationFunctionType.Exp
COPY = mybir.ActivationFunctionType.Copy
MULT = mybir.AluOpType.mult
ADD = mybir.AluOpType.add

# Schraudolph exp producing bf16 BITS via one DVE tensor_scalar:
# bf16_bits(e^x) ~= int16(x * 128/ln2 + (127<<7) - 0.0579*128).
EXP_A = 128.0 / float(np.log(2.0))
EXP_B = 16256.0 - 0.0579 * 128.0
# exp column split: ACT does [0:SPLITC], DVE does [SPLITC:1024] of each
# [128, 1024] score tile.
SPLITC = 640
NPBF16 = ml_dtypes.bfloat16


def build_nc():
    nc = bacc.Bacc()

    xT_d = nc.declare_dram_parameter("xT", [C, T], BF16, isOutput=False)
    wq_d = nc.declare_dram_parameter("wq", [C, DQ], BF16, isOutput=False)
    wk_d = nc.declare_dram_parameter("wk", [C, DKV], BF16, isOutput=False)
    wv_d = nc.declare_dram_parameter("wv", [C, DKV], BF16, isOutput=False)
    wo_d = nc.declare_dram_parameter("wo", [C, DQ], BF16, isOutput=False)
    cosr_d = nc.declare_dram_parameter("cosr", [128, T], BF16, isOutput=False)
    sinr_d = nc.declare_dram_parameter("sinr", [128, T], BF16, isOutput=False)
    out_d = nc.declare_dram_parameter("out", [T, DQ], F32, isOutput=True)

    with tile.TileContext(nc) as tc:
        with (
            tc.tile_pool(name="persist", bufs=1) as pp,
            tc.tile_pool(name="dram", bufs=1, space="DRAM") as dp,
        ):
            # ---------- persistent SBUF ----------
            # roped Q^T tiles: qt[p] holds local heads (2p, 2p+1) on partitions
            # [0:64] / [64:128]; free dim = t
            qt = [pp.tile([128, T], BF16, tag=f"qt{i}", name=f"qt{i}") for i in range(4)]
            # duplicated roped K^T tiles: ktd[j] = [kv_j ; kv_j]
            ktd = [pp.tile([128, T], BF16, tag=f"ktd{i}", name=f"ktd{i}") for i in range(2)]
            # V augmented with a ones column: per kv head, per s-tile [128, 65]
            vaug = [
                [pp.tile([128, HD + 1], BF16, tag=f"va{j}_{s}", name=f"va{j}_{s}") for s in range(16)]
                for j in range(2)
            ]
            cosr = pp.tile([128, T], BF16, tag="cosr")
            sinr = pp.tile([128, T], BF16, tag="sinr")
            wo_sb = [pp.tile([128, DQ], BF16, tag=f"wo{i}", name=f"wo{i}") for i in range(16)]

            for j in range(2):
                for s in range(16):
                    nc.gpsimd.memset(vaug[j][s][:, HD:HD + 1], 1.0)
            # warm the ACT exp table set early so the ~2.7us ACT_TABLE_LOAD is
            # off the attention critical path
            warm = pp.tile([1, 8], F32, tag="warm")
            nc.gpsimd.memset(warm[:], 0.0)
            nc.scalar.activation(warm[:], warm[:], EXP)

            # ---------- DRAM bounce for AllGather (4 chunks of 512 t) ----------
            cc_in = [dp.tile([DQ, 512], BF16, tag=f"cci{i}", name=f"cci{i}") for i in range(4)]
            cc_out = [dp.tile([4 * DQ, 512], BF16, tag=f"cco{i}", name=f"cco{i}") for i in range(4)]

            # warmup collective: absorbs the DGE start delay (~11us) and the
            # initial cross-core sync skew so the first real AllGather is fast
            cw_in = dp.tile([128, 16], BF16, tag="cwi", name="cwi")
            cw_out = dp.tile([512, 16], BF16, tag="cwo", name="cwo")

            # ================= Phase A: projections + RoPE + V =================
            with (
                tc.tile_pool(name="pa", bufs=1) as pa,
                tc.tile_pool(name="pa_ps", bufs=1, space=bass.MemorySpace.PSUM) as pps,
            ):
                # HAM warmup: keep the PE busy while the first DMAs land so
                # phase A's matmuls run at 2.4 GHz from the start.
                junk = pa.tile([128, 512], BF16, tag="junk")
                nc.gpsimd.memset(junk[:], 0.0)
                nc.sync.dma_start(out=cw_in[:], in_=junk[:, 0:16])
                nc.gpsimd.collective_compute(
                    "AllGather",
                    mybir.AluOpType.bypass,
                    replica_groups=[[0, 1, 2, 3], [4, 5, 6, 7]],
                    ins=[cw_in[:].opt()],
                    outs=[cw_out[:].opt()],
                )
                for _ in range(10):
                    jps = pps.tile([128, 512], F32, tag="proj", bufs=6)
                    nc.tensor.matmul(jps[:], junk[:, 0:128], junk[:], start=True, stop=True)

                wq_sb = [pa.tile([128, DQ], BF16, tag=f"wq{i}", name=f"wq{i}") for i in range(16)]
                wk_sb = [pa.tile([128, DKV], BF16, tag=f"wk{i}", name=f"wk{i}") for i in range(16)]
                wv_sb = [pa.tile([128, DKV], BF16, tag=f"wv{i}", name=f"wv{i}") for i in range(16)]

                # raw (pre-rope) projections, bf16 in SBUF
                qraw = [pa.tile([128, T], BF16, tag=f"qraw{i}", name=f"qraw{i}") for i in range(4)]
                ktraw = pa.tile([128, T], BF16, tag="ktraw")
                # x^T tiles: one tag per (kc, half) so half 1's DMAs start
                # immediately instead of waiting for half 0's last consumer
                xth = [
                    [
                        pa.tile([128, 1024], BF16, tag=f"xt{kc}_{h}", name=f"xt{kc}_{h}")
                        for kc in range(16)
                    ]
                    for h in range(2)
                ]

                # K/V weights first (K-proj unblocks earliest), then x^T half 0
                # interleaved with wq, then the rest.
                for kc in range(16):
                    nc.sync.dma_start(out=wk_sb[kc][:], in_=wk_d[kc * 128:(kc + 1) * 128, :])
                    nc.sync.dma_start(out=wv_sb[kc][:], in_=wv_d[kc * 128:(kc + 1) * 128, :])
                for kc in range(16):
                    nc.sync.dma_start(
                        out=xth[0][kc][:], in_=xT_d[kc * 128:(kc + 1) * 128, 0:1024]
                    )
                    nc.sync.dma_start(out=wq_sb[kc][:], in_=wq_d[kc * 128:(kc + 1) * 128, :])
                nc.sync.dma_start(out=cosr[:], in_=cosr_d[:])
                nc.sync.dma_start(out=sinr[:], in_=sinr_d[:])
                for kc in range(16):
                    nc.sync.dma_start(
                        out=xth[1][kc][:], in_=xT_d[kc * 128:(kc + 1) * 128, 1024:2048]
                    )
                for i in range(16):
                    nc.sync.dma_start(out=wo_sb[i][:], in_=wo_d[i * 128:(i + 1) * 128, :])

                # ---- RoPE on a [128, 1024] half: dest = raw*cosr + swap32(raw)*sinr ----
                def rope_half(raw, dest, t0):
                    swp = pa.tile([128, 1024], BF16, tag="swp", bufs=2)
                    for a, b in ((0, 32), (32, 0), (64, 96), (96, 64)):
                        nc.sync.dma_start(out=swp[a:a + 32, :], in_=raw[b:b + 32, t0:t0 + 1024])
                    t1 = pa.tile([128, 1024], BF16, tag="t1", bufs=2)
                    t2 = pa.tile([128, 1024], BF16, tag="t2", bufs=2)
                    nc.vector.tensor_mul(t1[:], raw[:, t0:t0 + 1024], cosr[:, t0:t0 + 1024])
                    nc.vector.tensor_mul(t2[:], swp[:], sinr[:, t0:t0 + 1024])
                    nc.vector.tensor_add(dest[:, t0:t0 + 1024], t1[:], t2[:])

                for half in range(2):
                    t0 = half * 1024
                    xt = xth[half]
                    # K^T tile first (only needs wk + this half's xT)
                    for ch in range(2):
                        ps = pps.tile([128, 512], F32, tag="proj", bufs=6)
                        for kc in range(16):
                            nc.tensor.matmul(
                                ps[:],
                                wk_sb[kc][:],
                                xt[kc][:, ch * 512:(ch + 1) * 512],
                                start=(kc == 0),
                                stop=(kc == 15),
                            )
                        nc.vector.tensor_copy(
                            ktraw[:, t0 + ch * 512:t0 + (ch + 1) * 512], ps[:]
                        )
                    # V in [s, d] layout: lhsT = xT tile slice (stationary), rhs = wv
                    for sl in range(8):
                        s = half * 8 + sl
                        psv = pps.tile([128, 128], F32, tag="vps", bufs=2)
                        for kc in range(16):
                            nc.tensor.matmul(
                                psv[:],
                                xt[kc][:, sl * 128:(sl + 1) * 128],
                                wv_sb[kc][:],
                                start=(kc == 0),
                                stop=(kc == 15),
                            )
                        nc.vector.tensor_copy(vaug[0][s][:, 0:HD], psv[:, 0:HD])
                        nc.vector.tensor_copy(vaug[1][s][:, 0:HD], psv[:, HD:2 * HD])
                    # K rope writes into a temp then duplicated halves of ktd
                    ktr = pa.tile([128, 1024], BF16, tag="ktr", bufs=2)
                    swp = pa.tile([128, 1024], BF16, tag="swpk", bufs=2)
                    for a, b in ((0, 32), (32, 0), (64, 96), (96, 64)):
                        nc.sync.dma_start(out=swp[a:a + 32, :], in_=ktraw[b:b + 32, t0:t0 + 1024])
                    t1k = pa.tile([128, 1024], BF16, tag="t1k", bufs=2)
                    t2k = pa.tile([128, 1024], BF16, tag="t2k", bufs=2)
                    nc.vector.tensor_mul(t1k[:], ktraw[:, t0:t0 + 1024], cosr[:, t0:t0 + 1024])
                    nc.vector.tensor_mul(t2k[:], swp[:], sinr[:, t0:t0 + 1024])
                    nc.vector.tensor_add(ktr[:], t1k[:], t2k[:])
                    nc.sync.dma_start(out=ktd[0][0:64, t0:t0 + 1024], in_=ktr[0:64, :])
                    nc.sync.dma_start(out=ktd[0][64:128, t0:t0 + 1024], in_=ktr[0:64, :])
                    nc.sync.dma_start(out=ktd[1][0:64, t0:t0 + 1024], in_=ktr[64:128, :])
                    nc.sync.dma_start(out=ktd[1][64:128, t0:t0 + 1024], in_=ktr[64:128, :])
                    # Q^T tiles: out [128 dq, 512 t] = wq_tile^T @ xT
                    for dq in range(4):
                        for ch in range(2):
                            ps = pps.tile([128, 512], F32, tag="proj", bufs=6)
                            for kc in range(16):
                                nc.tensor.matmul(
                                    ps[:],
                                    wq_sb[kc][:, dq * 128:(dq + 1) * 128],
                                    xt[kc][:, ch * 512:(ch + 1) * 512],
                                    start=(kc == 0),
                                    stop=(kc == 15),
                                )
                            nc.vector.tensor_copy(
                                qraw[dq][:, t0 + ch * 512:t0 + (ch + 1) * 512], ps[:]
                            )
                    # RoPE for this half
                    for dq in range(4):
                        rope_half(qraw[dq], qt[dq], t0)

            # ================= Phase B: attention + AG + wo =================
            with (
                tc.tile_pool(name="pb", bufs=1) as pb,
                tc.tile_pool(name="pb_ps", bufs=1, space=bass.MemorySpace.PSUM) as bps,
            ):
                ag_sb = [
                    pp.tile([128, 512], BF16, tag=f"ag{d}", name=f"ag{d}")
                    for d in range(16)
                ]

                def new_ctx(chunk, pair):
                    return {
                        "chunk": chunk, "pair": pair,
                        "ta": chunk * 512, "kv": pair // 2,
                        "pv_a": bps.tile([HD + 1, 512], F32, tag="pv", bufs=2, name="pv_a"),
                        "pv_b": bps.tile([HD + 1, 512], F32, tag="pv", bufs=2, name="pv_b"),
                        "qks": {}, "ess": {},
                    }

                def emit_qk(ctx, s):
                    qk = bps.tile([128, 1024], F32, tag="qk", bufs=3)
                    ctx["qks"][s] = qk
                    kv, pair, ta = ctx["kv"], ctx["pair"], ctx["ta"]
                    # row-packed pair: head A on rows 0-63 -> tile (0,0),
                    # head B on rows 64-127 -> tile (64,0): concurrent MMs
                    nc.tensor.matmul(
                        qk[:, 0:512],
                        ktd[kv][0:64, s * 128:(s + 1) * 128],
                        qt[pair][0:64, ta:ta + 512],
                        start=True, stop=True,
                    )
                    nc.tensor.matmul(
                        qk[:, 512:1024],
                        ktd[kv][64:128, s * 128:(s + 1) * 128],
                        qt[pair][64:128, ta:ta + 512],
                        start=True, stop=True,
                    )

                def emit_exp(ctx, s):
                    # full-tile exp, alternating engines per s so each
                    # engine pays one dispatch bubble per 1024 columns:
                    # even s on ACT, odd s on DVE (Schraudolph bf16-bits).
                    # bufs=4 (even) so each pool slot is always rewritten
                    # by the SAME engine -> no cross-engine writer deps.
                    qk = ctx["qks"][s]
                    es = pb.tile([128, 1024], BF16, tag="es", bufs=4)
                    ctx["ess"][s] = es
                    if s % 2 == 0:
                        nc.scalar.activation(es[:], qk[:], EXP)
                    else:
                        nc.vector.tensor_scalar(
                            es[:].bitcast(I16), qk[:],
                            EXP_A, EXP_B, MULT, ADD,
                        )

                def emit_pv(ctx, s):
                    es = ctx["ess"].pop(s)
                    ctx["qks"].pop(s)
                    kv = ctx["kv"]
                    nc.tensor.matmul(
                        ctx["pv_a"][:], vaug[kv][s][:], es[:, 0:512],
                        start=(s == 0), stop=(s == 15),
                        skip_group_check=True,
                    )
                    nc.tensor.matmul(
                        ctx["pv_b"][:], vaug[kv][s][:], es[:, 512:1024],
                        start=(s == 0), stop=(s == 15),
                        skip_group_check=True,
                    )

                def emit_prologue(ctx):
                    # 2-deep lookahead: QK runs two iterations ahead of PV so
                    # the exp latency never lands on the PE critical path
                    emit_qk(ctx, 0)
                    emit_qk(ctx, 1)
                    emit_exp(ctx, 0)

                def emit_body(ctx):
                    for s in range(16):
                        if s + 2 < 16:
                            emit_qk(ctx, s + 2)
                        if s + 1 < 16:
                            emit_exp(ctx, s + 1)
                        emit_pv(ctx, s)

                def emit_tail(ctx):
                    chunk, pair = ctx["chunk"], ctx["pair"]
                    pv_a, pv_b = ctx["pv_a"], ctx["pv_b"]
                    # --- evacuate pv (frees the PSUM banks for the next pair:
                    # ACT takes head A, DVE takes head B, in parallel) ---
                    pvs_a = pb.tile([HD + 1, 512], F32, tag="pvsa", bufs=2)
                    pvs_b = pb.tile([HD + 1, 512], F32, tag="pvsb", bufs=2)
                    nc.scalar.activation(pvs_a[:], pv_a[:], COPY)
                    nc.vector.tensor_copy(pvs_b[:], pv_b[:])
                    # --- denominator -> reciprocal -> normalize -> cc_in ---
                    # den rows must reach partition 0 for the gpsimd broadcast
                    denr = pb.tile([1, 1024], F32, tag="denr", bufs=2)
                    nc.sync.dma_start(out=denr[0:1, 0:512], in_=pvs_a[64:65, :])
                    nc.sync.dma_start(out=denr[0:1, 512:1024], in_=pvs_b[64:65, :])
                    denb = pb.tile([64, 1024], F32, tag="denb", bufs=2)
                    nc.gpsimd.partition_broadcast(denb[:], denr[0:1, :], channels=64)
                    rep = pb.tile([64, 1024], F32, tag="rep", bufs=2)
                    nc.vector.reciprocal_approx_fast(out=rep[:], in_=denb[:])
                    tma = pb.tile([64, 512], BF16, tag="tma", bufs=2)
                    tmb = pb.tile([64, 512], BF16, tag="tmb", bufs=2)
                    nc.gpsimd.tensor_mul(tma[:], pvs_a[0:64, :], rep[:, 0:512])
                    nc.gpsimd.tensor_mul(tmb[:], pvs_b[0:64, :], rep[:, 512:1024])
                    nc.sync.dma_start(
                        out=cc_in[chunk][pair * 128:pair * 128 + 64, :], in_=tma[:]
                    )
                    nc.sync.dma_start(
                        out=cc_in[chunk][pair * 128 + 64:pair * 128 + 128, :], in_=tmb[:]
                    )

                def do_ag_chunk(chunk):
                    nc.gpsimd.collective_compute(
                        "AllGather",
                        mybir.AluOpType.bypass,
                        replica_groups=[[0, 1, 2, 3], [4, 5, 6, 7]],
                        ins=[cc_in[chunk][:].opt()],
                        outs=[cc_out[chunk][:].opt()],
                    )

                def emit_ag_loads(chunk):
                    for d in range(16):
                        nc.sync.dma_start(
                            out=ag_sb[d][:], in_=cc_out[chunk][d * 128:(d + 1) * 128, :]
                        )

                def emit_wo_group(chunk, tt):
                    """one [128, 512] psum tile of out[:, chunk cols]"""
                    tb = chunk * 512
                    pso = bps.tile([128, 512], F32, tag="qk", bufs=3)
                    for d in range(16):
                        nc.tensor.matmul(
                            pso[:],
                            ag_sb[d][:, tt * 128:(tt + 1) * 128],
                            wo_sb[d][:],
                            start=(d == 0),
                            stop=(d == 15),
                        )
                    osb = pb.tile([128, 512], F32, tag="osb", bufs=2)
                    nc.scalar.activation(osb[:], pso[:], COPY)
                    nc.sync.dma_start(
                        out=out_d[tb + tt * 128:tb + (tt + 1) * 128, :], in_=osb[:]
                    )

                # schedule: per pair, the NEXT pair's pipeline prologue is
                # emitted before this pair's den/normalize tail, so the exp
                # pipeline refills while the tail runs; wo groups of chunk c-1
                # sit before pairs 2/3 (AG(c-1) needs ~25us) plus two right
                # after AG(c) is issued, bridging the chunk boundary.
                pending = {}
                prev = None
                for chunk in range(4):
                    if chunk >= 1:
                        emit_ag_loads(chunk - 1)
                    for pair in range(4):
                        ctx = pending.pop((chunk, pair), None)
                        if ctx is None:
                            ctx = new_ctx(chunk, pair)
                            emit_prologue(ctx)
                        if prev is not None:
                            emit_tail(prev)
                            prev = None
                        if chunk >= 1 and pair >= 2:
                            emit_wo_group(chunk - 1, pair - 2)
                        emit_body(ctx)
                        prev = ctx
                    if chunk < 3:
                        nctx = new_ctx(chunk + 1, 0)
                        emit_prologue(nctx)
                        pending[(chunk + 1, 0)] = nctx
                    emit_tail(prev)
                    prev = None
                    do_ag_chunk(chunk)
                    if chunk >= 1:
                        emit_wo_group(chunk - 1, 2)
                        emit_wo_group(chunk - 1, 3)
                emit_ag_loads(3)
                for tt in range(4):
                    emit_wo_group(3, tt)

    return nc


# ---------------------------------------------------------------------------
# Host side
# ---------------------------------------------------------------------------

_CACHE = {}


def _rope_tables():
    i = np.arange(32)
    freqs = 1.0 / (THETA ** (2.0 * i / HD))          # [32]
    ang = np.arange(T, dtype=np.float64)[:, None] * freqs[None, :]  # [T, 32]
    cos = np.cos(ang)
    sin = np.sin(ang)
    p = np.arange(128)
    fi = p % 32
    sign = np.where(p % 64 < 32, -1.0, 1.0)
    cosr = cos[:, fi].T                               # [128, T]
    sinr = (sin[:, fi] * sign[None, :]).T             # [128, T]
    return cosr.astype(np.float32), sinr.astype(np.float32)


def _colperm(n_heads):
    """rotate-half permutation: per 64-col head block, evens then odds"""
    blk = np.concatenate([np.arange(0, HD, 2), np.arange(1, HD, 2)])
    return np.concatenate([h * HD + blk for h in range(n_heads)])


def _prep_in_maps(x, wq, wk, wv, wo):
    cosr, sinr = _rope_tables()
    qperm = _colperm(32)
    kperm = _colperm(8)
    wq_p = (wq.astype(np.float64) / 8.0)[:, qperm]    # fold 1/sqrt(hd)
    wk_p = wk[:, kperm]
    in_maps = []
    for c in range(N_CORES):
        b, g = divmod(c, 4)
        in_maps.append({
            "xT": np.ascontiguousarray(x[b].T).astype(NPBF16),
            "wq": wq_p[:, g * DQ:(g + 1) * DQ].astype(NPBF16),
            "wk": wk_p[:, g * DKV:(g + 1) * DKV].astype(NPBF16),
            "wv": wv[:, g * DKV:(g + 1) * DKV].astype(NPBF16),
            "wo": wo[:, g * DQ:(g + 1) * DQ].astype(NPBF16),
            "cosr": cosr.astype(NPBF16),
            "sinr": sinr.astype(NPBF16),
        })
    return in_maps


def get_nc():
    if "nc" not in _CACHE:
        nc = build_nc()
        if not nc.is_finalized():
            nc.finalize()
        _CACHE["nc"] = nc
    return _CACHE["nc"]


def run_on_hw(in_maps, trace=False):
    nc = get_nc()
    return run_bass_kernel_spmd(nc, in_maps, core_ids=list(range(N_CORES)), trace=trace)


def _assemble(results):
    out = np.zeros((2, T, C), dtype=np.float32)
    for c in range(N_CORES):
        b, g = divmod(c, 4)
        out[b][:, g * DQ:(g + 1) * DQ] = np.asarray(results[c]["out"], dtype=np.float32)
    return out


def kernel(x, wq, wk, wv, wo):
    in_maps = _prep_in_maps(
        np.asarray(x, np.float32), np.asarray(wq, np.float32),
        np.asarray(wk, np.float32), np.asarray(wv, np.float32),
        np.asarray(wo, np.float32),
    )
    res = run_on_hw(in_maps, trace=False)
    return _assemble(res.results)


# revision 19
# speedup vs baseline: 1.3053x; 1.3053x over previous
"""Trainium2 Bass kernel for GQA attention with RoPE (dense_transformer).

Reference computation (per batch b):
    q = x @ wq  -> [T, 32, 64],  k = x @ wk -> [T, 8, 64], v = x @ wv
    rope(q), rope(k); scores = q k^T / 8; w = softmax(scores); out = (w v) @ wo

Sharding over 8 NeuronCores: 2 batch groups x 4-way head tensor parallel.
Core c: batch b=c//4, head group g=c%4 (q-heads 8g..8g+8, kv-heads 2g,2g+1).
Within a group of 4 cores the attention outputs (transposed, [512,T]) are
AllGather'd per 512-column t-chunk; each core then computes a 512-column
slice of out = attn @ wo.

v2 schedule (single pipeline, engine-balanced):
  - Phase A: projections + RoPE with HAM-warmup matmuls and DMA-chased
    accumulation.  Q/K/V all projected up front; qt/ktd/vaug persist.
  - Phase B: per 512-col t-chunk, per head-pair: software-pipelined s-loop
    emitting QK(s+1) -> exp(s) -> PV(s).  QK pairs are row-tiled (K=64 on
    partitions 0:64/64:128) so both heads' score matmuls run concurrently.
  - exp is column-split across engines per tile: ACT handles cols
    [0:SPLITC], DVE handles [SPLITC:1024] via a Schraudolph bf16-bits
    tensor_scalar (int16(x*128/ln2 + const) == bf16 bits of e^x, ~1.5%
    elementwise, cancels via the softmax denominator common mode).
  - The softmax denominator is the 65th (ones) column of the V stationary,
    so it falls out of the PV matmul for free; normalization happens at
    PSUM->SBUF evacuation (DVE muls by a gpsimd-broadcast reciprocal).
  - wo matmul groups for chunk c-1 are emitted between chunk c's pairs:
    they fill the PE while ACT/DVE/gpsimd run the den/normalize tail, and
    the AllGather latency of chunk c-1 hides under chunk c's attention.
"""

import numpy as np
import ml_dtypes

import concourse.bass as bass
import concourse.mybir as mybir
import concourse.tile as tile
from concourse import bacc
from concourse.bass_utils import run_bass_kernel_spmd

BF16 = mybir.dt.bfloat16
F32 = mybir.dt.float32
I16 = mybir.dt.int16

T = 2048          # sequence length (also s dim)
C = 2048          # model dim
HD = 64           # head dim
DQ = 512          # q dims per core (8 heads)
DKV = 128         # kv dims per core (2 kv heads)
N_CORES = 8
THETA = 10000.0

EXP = mybir.ActivationFunctionType.Exp
COPY = mybir.ActivationFunctionType.Copy
MULT = mybir.AluOpType.mult
ADD = mybir.AluOpType.add

# Schraudolph exp producing bf16 BITS via one DVE tensor_scalar:
# bf16_bits(e^x) ~= int16(x * 128/ln2 + (127<<7) - 0.0579*128).
EXP_A = 128.0 / float(np.log(2.0))
EXP_B = 16256.0 - 0.0579 * 128.0
# exp column split: ACT does [0:SPLITC], DVE does [SPLITC:1024] of each
# [128, 1024] score tile.
SPLITC = 640
NPBF16 = ml_dtypes.bfloat16


def build_nc():
    nc = bacc.Bacc()

    xT_d = nc.declare_dram_parameter("xT", [C, T], BF16, isOutput=False)
    wq_d = nc.declare_dram_parameter("wq", [C, DQ], BF16, isOutput=False)
    wk_d = nc.declare_dram_parameter("wk", [C, DKV], BF16, isOutput=False)
    wv_d = nc.declare_dram_parameter("wv", [C, DKV], BF16, isOutput=False)
    wo_d = nc.declare_dram_parameter("wo", [C, DQ], BF16, isOutput=False)
    cosr_d = nc.declare_dram_parameter("cosr", [128, T], BF16, isOutput=False)
    sinr_d = nc.declare_dram_parameter("sinr", [128, T], BF16, isOutput=False)
    out_d = nc.declare_dram_parameter("out", [T, DQ], F32, isOutput=True)

    with tile.TileContext(nc) as tc:
        with (
            tc.tile_pool(name="persist", bufs=1) as pp,
            tc.tile_pool(name="dram", bufs=1, space="DRAM") as dp,
        ):
            # ---------- persistent SBUF ----------
            # roped Q^T tiles: qt[p] holds local heads (2p, 2p+1) on partitions
            # [0:64] / [64:128]; free dim = t
            qt = [pp.tile([128, T], BF16, tag=f"qt{i}", name=f"qt{i}") for i in range(4)]
            # duplicated roped K^T tiles: ktd[j] = [kv_j ; kv_j]
            ktd = [pp.tile([128, T], BF16, tag=f"ktd{i}", name=f"ktd{i}") for i in range(2)]
            # V augmented with a ones column: per kv head, per s-tile [128, 65]
            vaug = [
                [pp.tile([128, HD + 1], BF16, tag=f"va{j}_{s}", name=f"va{j}_{s}") for s in range(16)]
                for j in range(2)
            ]
            cosr = pp.tile([128, T], BF16, tag="cosr")
            sinr = pp.tile([128, T], BF16, tag="sinr")
            wo_sb = [pp.tile([128, DQ], BF16, tag=f"wo{i}", name=f"wo{i}") for i in range(16)]

            for j in range(2):
                for s in range(16):
                    nc.gpsimd.memset(vaug[j][s][:, HD:HD + 1], 1.0)
            # warm the ACT exp table set early so the ~2.7us ACT_TABLE_LOAD is
            # off the attention critical path
            warm = pp.tile([1, 8], F32, tag="warm")
            nc.gpsimd.memset(warm[:], 0.0)
            nc.scalar.activation(warm[:], warm[:], EXP)

            # ---------- DRAM bounce for AllGather (4 chunks of 512 t) ----------
            cc_in = [dp.tile([DQ, 512], BF16, tag=f"cci{i}", name=f"cci{i}") for i in range(4)]
            cc_out = [dp.tile([4 * DQ, 512], BF16, tag=f"cco{i}", name=f"cco{i}") for i in range(4)]

            # warmup collective: absorbs the DGE start delay (~11us) and the
            # initial cross-core sync skew so the first real AllGather is fast
            cw_in = dp.tile([128, 16], BF16, tag="cwi", name="cwi")
            cw_out = dp.tile([512, 16], BF16, tag="cwo", name="cwo")

            # ================= Phase A: projections + RoPE + V =================
            with (
                tc.tile_pool(name="pa", bufs=1) as pa,
                tc.tile_pool(name="pa_ps", bufs=1, space=bass.MemorySpace.PSUM) as pps,
            ):
                # HAM warmup: keep the PE busy while the first DMAs land so
                # phase A's matmuls run at 2.4 GHz from the start.
                junk = pa.tile([128, 512], BF16, tag="junk")
                nc.gpsimd.memset(junk[:], 0.0)
                nc.sync.dma_start(out=cw_in[:], in_=junk[:, 0:16])
                nc.gpsimd.collective_compute(
                    "AllGather",
                    mybir.AluOpType.bypass,
                    replica_groups=[[0, 1, 2, 3], [4, 5, 6, 7]],
                    ins=[cw_in[:].opt()],
                    outs=[cw_out[:].opt()],
                )
                for _ in range(10):
                    jps = pps.tile([128, 512], F32, tag="proj", bufs=6)
                    nc.tensor.matmul(jps[:], junk[:, 0:128], junk[:], start=True, stop=True)

                wq_sb = [pa.tile([128, DQ], BF16, tag=f"wq{i}", name=f"wq{i}") for i in range(16)]
                wk_sb = [pa.tile([128, DKV], BF16, tag=f"wk{i}", name=f"wk{i}") for i in range(16)]
                wv_sb = [pa.tile([128, DKV], BF16, tag=f"wv{i}", name=f"wv{i}") for i in range(16)]

                # raw (pre-rope) projections, bf16 in SBUF
                qraw = [pa.tile([128, T], BF16, tag=f"qraw{i}", name=f"qraw{i}") for i in range(4)]
                ktraw = pa.tile([128, T], BF16, tag="ktraw")
                # x^T tiles: one tag per (kc, half) so half 1's DMAs start
                # immediately instead of waiting for half 0's last consumer
                xth = [
                    [
                        pa.tile([128, 1024], BF16, tag=f"xt{kc}_{h}", name=f"xt{kc}_{h}")
                        for kc in range(16)
                    ]
                    for h in range(2)
                ]

                # K/V weights first (K-proj unblocks earliest), then x^T half 0
                # interleaved with wq, then the rest.
                for kc in range(16):
                    nc.sync.dma_start(out=wk_sb[kc][:], in_=wk_d[kc * 128:(kc + 1) * 128, :])
                    nc.sync.dma_start(out=wv_sb[kc][:], in_=wv_d[kc * 128:(kc + 1) * 128, :])
                for kc in range(16):
                    nc.sync.dma_start(
                        out=xth[0][kc][:], in_=xT_d[kc * 128:(kc + 1) * 128, 0:1024]
                    )
                    nc.sync.dma_start(out=wq_sb[kc][:], in_=wq_d[kc * 128:(kc + 1) * 128, :])
                nc.sync.dma_start(out=cosr[:], in_=cosr_d[:])
                nc.sync.dma_start(out=sinr[:], in_=sinr_d[:])
                for kc in range(16):
                    nc.sync.dma_start(
                        out=xth[1][kc][:], in_=xT_d[kc * 128:(kc + 1) * 128, 1024:2048]
                    )
                for i in range(16):
                    nc.sync.dma_start(out=wo_sb[i][:], in_=wo_d[i * 128:(i + 1) * 128, :])

                # ---- RoPE on a [128, 1024] half: dest = raw*cosr + swap32(raw)*sinr ----
                def rope_half(raw, dest, t0):
                    swp = pa.tile([128, 1024], BF16, tag="swp", bufs=2)
                    for a, b in ((0, 32), (32, 0), (64, 96), (96, 64)):
                        nc.sync.dma_start(out=swp[a:a + 32, :], in_=raw[b:b + 32, t0:t0 + 1024])
                    t1 = pa.tile([128, 1024], BF16, tag="t1", bufs=2)
                    t2 = pa.tile([128, 1024], BF16, tag="t2", bufs=2)
                    nc.vector.tensor_mul(t1[:], raw[:, t0:t0 + 1024], cosr[:, t0:t0 + 1024])
                    nc.vector.tensor_mul(t2[:], swp[:], sinr[:, t0:t0 + 1024])
                    nc.vector.tensor_add(dest[:, t0:t0 + 1024], t1[:], t2[:])

                for half in range(2):
                    t0 = half * 1024
                    xt = xth[half]
                    # K^T tile first (only needs wk + this half's xT)
                    for ch in range(2):
                        ps = pps.tile([128, 512], F32, tag="proj", bufs=6)
                        for kc in range(16):
                            nc.tensor.matmul(
                                ps[:],
                                wk_sb[kc][:],
                                xt[kc][:, ch * 512:(ch + 1) * 512],
                                start=(kc == 0),
                                stop=(kc == 15),
                            )
                        nc.vector.tensor_copy(
                            ktraw[:, t0 + ch * 512:t0 + (ch + 1) * 512], ps[:]
                        )
                    # V in [s, d] layout: lhsT = xT tile slice (stationary), rhs = wv
                    for sl in range(8):
                        s = half * 8 + sl
                        psv = pps.tile([128, 128], F32, tag="vps", bufs=2)
                        for kc in range(16):
                            nc.tensor.matmul(
                                psv[:],
                                xt[kc][:, sl * 128:(sl + 1) * 128],
                                wv_sb[kc][:],
                                start=(kc == 0),
                                stop=(kc == 15),
                            )
                        nc.vector.tensor_copy(vaug[0][s][:, 0:HD], psv[:, 0:HD])
                        nc.vector.tensor_copy(vaug[1][s][:, 0:HD], psv[:, HD:2 * HD])
                    # K rope writes into a temp then duplicated halves of ktd
                    ktr = pa.tile([128, 1024], BF16, tag="ktr", bufs=2)
                    swp = pa.tile([128, 1024], BF16, tag="swpk", bufs=2)
                    for a, b in ((0, 32), (32, 0), (64, 96), (96, 64)):
                        nc.sync.dma_start(out=swp[a:a + 32, :], in_=ktraw[b:b + 32, t0:t0 + 1024])
                    t1k = pa.tile([128, 1024], BF16, tag="t1k", bufs=2)
                    t2k = pa.tile([128, 1024], BF16, tag="t2k", bufs=2)
                    nc.vector.tensor_mul(t1k[:], ktraw[:, t0:t0 + 1024], cosr[:, t0:t0 + 1024])
                    nc.vector.tensor_mul(t2k[:], swp[:], sinr[:, t0:t0 + 1024])
                    nc.vector.tensor_add(ktr[:], t1k[:], t2k[:])
                    nc.sync.dma_start(out=ktd[0][0:64, t0:t0 + 1024], in_=ktr[0:64, :])
                    nc.sync.dma_start(out=ktd[0][64:128, t0:t0 + 1024], in_=ktr[0:64, :])
                    nc.sync.dma_start(out=ktd[1][0:64, t0:t0 + 1024], in_=ktr[64:128, :])
                    nc.sync.dma_start(out=ktd[1][64:128, t0:t0 + 1024], in_=ktr[64:128, :])
                    # Q^T tiles: out [128 dq, 512 t] = wq_tile^T @ xT
                    for dq in range(4):
                        for ch in range(2):
                            ps = pps.tile([128, 512], F32, tag="proj", bufs=6)
                            for kc in range(16):
                                nc.tensor.matmul(
                                    ps[:],
                                    wq_sb[kc][:, dq * 128:(dq + 1) * 128],
                                    xt[kc][:, ch * 512:(ch + 1) * 512],
                                    start=(kc == 0),
                                    stop=(kc == 15),
                                )
                            nc.vector.tensor_copy(
                                qraw[dq][:, t0 + ch * 512:t0 + (ch + 1) * 512], ps[:]
                            )
                    # RoPE for this half
                    for dq in range(4):
                        rope_half(qraw[dq], qt[dq], t0)

            # ================= Phase B: attention + AG + wo =================
            with (
                tc.tile_pool(name="pb", bufs=1) as pb,
                tc.tile_pool(name="pb_ps", bufs=1, space=bass.MemorySpace.PSUM) as bps,
            ):
                ag_sb = [
                    pp.tile([128, 512], BF16, tag=f"ag{d}", name=f"ag{d}")
                    for d in range(16)
                ]

                def new_ctx(chunk, pair):
                    return {
                        "chunk": chunk, "pair": pair,
                        "ta": chunk * 512, "kv": pair // 2,
                        "pv_a": bps.tile([HD + 1, 512], F32, tag="pv", bufs=2, name="pv_a"),
                        "pv_b": bps.tile([HD + 1, 512], F32, tag="pv", bufs=2, name="pv_b"),
                        "qks": {}, "ess": {},
                    }

                def emit_qk(ctx, s):
                    qk = bps.tile([128, 1024], F32, tag="qk", bufs=3)
                    ctx["qks"][s] = qk
                    kv, pair, ta = ctx["kv"], ctx["pair"], ctx["ta"]
                    # row-packed pair: head A on rows 0-63 -> tile (0,0),
                    # head B on rows 64-127 -> tile (64,0): concurrent MMs
                    nc.tensor.matmul(
                        qk[:, 0:512],
                        ktd[kv][0:64, s * 128:(s + 1) * 128],
                        qt[pair][0:64, ta:ta + 512],
                        start=True, stop=True,
                    )
                    nc.tensor.matmul(
                        qk[:, 512:1024],
                        ktd[kv][64:128, s * 128:(s + 1) * 128],
                        qt[pair][64:128, ta:ta + 512],
                        start=True, stop=True,
                    )

                def emit_exp(ctx, s):
                    # full-tile exp, alternating engines per s so each
                    # engine pays one dispatch bubble per 1024 columns:
                    # even s on ACT, odd s on DVE (Schraudolph bf16-bits).
                    # bufs=4 (even) so each pool slot is always rewritten
                    # by the SAME engine -> no cross-engine writer deps.
                    qk = ctx["qks"][s]
                    es = pb.tile([128, 1024], BF16, tag="es", bufs=4)
                    ctx["ess"][s] = es
                    if s % 2 == 0:
                        nc.scalar.activation(es[:], qk[:], EXP)
                    else:
                        nc.vector.tensor_scalar(
                            es[:].bitcast(I16), qk[:],
                            EXP_A, EXP_B, MULT, ADD,
                        )

                def emit_pv(ctx, s):
                    es = ctx["ess"].pop(s)
                    ctx["qks"].pop(s)
                    kv = ctx["kv"]
                    nc.tensor.matmul(
                        ctx["pv_a"][:], vaug[kv][s][:], es[:, 0:512],
                        start=(s == 0), stop=(s == 15),
                        skip_group_check=True,
                    )
                    nc.tensor.matmul(
                        ctx["pv_b"][:], vaug[kv][s][:], es[:, 512:1024],
                        start=(s == 0), stop=(s == 15),
                        skip_group_check=True,
                    )

                def emit_prologue(ctx):
                    # 2-deep lookahead: QK runs two iterations ahead of PV so
                    # the exp latency never lands on the PE critical path
                    emit_qk(ctx, 0)
                    emit_qk(ctx, 1)
                    emit_exp(ctx, 0)

                def emit_body(ctx):
                    for s in range(16):
                        if s + 2 < 16:
                            emit_qk(ctx, s + 2)
                        if s + 1 < 16:
                            emit_exp(ctx, s + 1)
                        emit_pv(ctx, s)

                def emit_tail(ctx):
                    chunk, pair = ctx["chunk"], ctx["pair"]
                    pv_a, pv_b = ctx["pv_a"], ctx["pv_b"]
                    # --- evacuate pv (frees the PSUM banks for the next pair:
                    # ACT takes head A, DVE takes head B, in parallel) ---
                    pvs_a = pb.tile([HD + 1, 512], F32, tag="pvsa", bufs=2)
                    pvs_b = pb.tile([HD + 1, 512], F32, tag="pvsb", bufs=2)
                    nc.scalar.activation(pvs_a[:], pv_a[:], COPY)
                    nc.vector.tensor_copy(pvs_b[:], pv_b[:])
                    # --- denominator -> reciprocal -> normalize -> cc_in ---
                    # den rows must reach partition 0 for the gpsimd broadcast
                    denr = pb.tile([1, 1024], F32, tag="denr", bufs=2)
                    nc.sync.dma_start(out=denr[0:1, 0:512], in_=pvs_a[64:65, :])
                    nc.sync.dma_start(out=denr[0:1, 512:1024], in_=pvs_b[64:65, :])
                    denb = pb.tile([64, 1024], F32, tag="denb", bufs=2)
                    nc.gpsimd.partition_broadcast(denb[:], denr[0:1, :], channels=64)
                    rep = pb.tile([64, 1024], F32, tag="rep", bufs=2)
                    nc.vector.reciprocal_approx_fast(out=rep[:], in_=denb[:])
                    tma = pb.tile([64, 512], BF16, tag="tma", bufs=2)
                    tmb = pb.tile([64, 512], BF16, tag="tmb", bufs=2)
                    nc.vector.tensor_mul(tma[:], pvs_a[0:64, :], rep[:, 0:512])
                    nc.vector.tensor_mul(tmb[:], pvs_b[0:64, :], rep[:, 512:1024])
                    nc.sync.dma_start(
                        out=cc_in[chunk][pair * 128:pair * 128 + 64, :], in_=tma[:]
                    )
                    nc.sync.dma_start(
                        out=cc_in[chunk][pair * 128 + 64:pair * 128 + 128, :], in_=tmb[:]
                    )

                def do_ag_chunk(chunk):
                    nc.gpsimd.collective_compute(
                        "AllGather",
                        mybir.AluOpType.bypass,
                        replica_groups=[[0, 1, 2, 3], [4, 5, 6, 7]],
                        ins=[cc_in[chunk][:].opt()],
                        outs=[cc_out[chunk][:].opt()],
                    )

                def emit_ag_loads(chunk):
                    for d in range(16):
                        nc.sync.dma_start(
                            out=ag_sb[d][:], in_=cc_out[chunk][d * 128:(d + 1) * 128, :]
                        )

                def emit_wo_group(chunk, tt):
                    """one [128, 512] psum tile of out[:, chunk cols]"""
                    tb = chunk * 512
                    pso = bps.tile([128, 512], F32, tag="qk", bufs=3)
                    for d in range(16):
                        nc.tensor.matmul(
                            pso[:],
                            ag_sb[d][:, tt * 128:(tt + 1) * 128],
                            wo_sb[d][:],
                            start=(d == 0),
                            stop=(d == 15),
                        )
                    osb = pb.tile([128, 512], F32, tag="osb", bufs=2)
                    nc.scalar.activation(osb[:], pso[:], COPY)
                    nc.sync.dma_start(
                        out=out_d[tb + tt * 128:tb + (tt + 1) * 128, :], in_=osb[:]
                    )

                # schedule: per pair, the NEXT pair's pipeline prologue is
                # emitted before this pair's den/normalize tail, so the exp
                # pipeline refills while the tail runs; wo groups of chunk c-1
                # sit before pairs 2/3 (AG(c-1) needs ~25us) plus two right
                # after AG(c) is issued, bridging the chunk boundary.
                pending = {}
                prev = None
                for chunk in range(4):
                    if chunk >= 1:
                        emit_ag_loads(chunk - 1)
                    for pair in range(4):
                        ctx = pending.pop((chunk, pair), None)
                        if ctx is None:
                            ctx = new_ctx(chunk, pair)
                            emit_prologue(ctx)
                        if prev is not None:
                            emit_tail(prev)
                            prev = None
                        if chunk >= 1 and pair >= 2:
                            emit_wo_group(chunk - 1, pair - 2)
                        emit_body(ctx)
                        prev = ctx
                    if chunk < 3:
                        nctx = new_ctx(chunk + 1, 0)
                        emit_prologue(nctx)
                        pending[(chunk + 1, 0)] = nctx
                    emit_tail(prev)
                    prev = None
                    do_ag_chunk(chunk)
                    if chunk >= 1:
                        emit_wo_group(chunk - 1, 2)
                        emit_wo_group(chunk - 1, 3)
                emit_ag_loads(3)
                for tt in range(4):
                    emit_wo_group(3, tt)

    return nc


# ---------------------------------------------------------------------------
# Host side
# ---------------------------------------------------------------------------

_CACHE = {}


def _rope_tables():
    i = np.arange(32)
    freqs = 1.0 / (THETA ** (2.0 * i / HD))          # [32]
    ang = np.arange(T, dtype=np.float64)[:, None] * freqs[None, :]  # [T, 32]
    cos = np.cos(ang)
    sin = np.sin(ang)
    p = np.arange(128)
    fi = p % 32
    sign = np.where(p % 64 < 32, -1.0, 1.0)
    cosr = cos[:, fi].T                               # [128, T]
    sinr = (sin[:, fi] * sign[None, :]).T             # [128, T]
    return cosr.astype(np.float32), sinr.astype(np.float32)


def _colperm(n_heads):
    """rotate-half permutation: per 64-col head block, evens then odds"""
    blk = np.concatenate([np.arange(0, HD, 2), np.arange(1, HD, 2)])
    return np.concatenate([h * HD + blk for h in range(n_heads)])


def _prep_in_maps(x, wq, wk, wv, wo):
    cosr, sinr = _rope_tables()
    qperm = _colperm(32)
    kperm = _colperm(8)
    wq_p = (wq.astype(np.float64) / 8.0)[:, qperm]    # fold 1/sqrt(hd)
    wk_p = wk[:, kperm]
    in_maps = []
    for c in range(N_CORES):
        b, g = divmod(c, 4)
        in_maps.append({
            "xT": np.ascontiguousarray(x[b].T).astype(NPBF16),
            "wq": wq_p[:, g * DQ:(g + 1) * DQ].astype(NPBF16),
            "wk": wk_p[:, g * DKV:(g + 1) * DKV].astype(NPBF16),
            "wv": wv[:, g * DKV:(g + 1) * DKV].astype(NPBF16),
            "wo": wo[:, g * DQ:(g + 1) * DQ].astype(NPBF16),
            "cosr": cosr.astype(NPBF16),
            "sinr": sinr.astype(NPBF16),
        })
    return in_maps


def get_nc():
    if "nc" not in _CACHE:
        nc = build_nc()
        if not nc.is_finalized():
            nc.finalize()
        _CACHE["nc"] = nc
    return _CACHE["nc"]


def run_on_hw(in_maps, trace=False):
    nc = get_nc()
    return run_bass_kernel_spmd(nc, in_maps, core_ids=list(range(N_CORES)), trace=trace)


def _assemble(results):
    out = np.zeros((2, T, C), dtype=np.float32)
    for c in range(N_CORES):
        b, g = divmod(c, 4)
        out[b][:, g * DQ:(g + 1) * DQ] = np.asarray(results[c]["out"], dtype=np.float32)
    return out


def kernel(x, wq, wk, wv, wo):
    in_maps = _prep_in_maps(
        np.asarray(x, np.float32), np.asarray(wq, np.float32),
        np.asarray(wk, np.float32), np.asarray(wv, np.float32),
        np.asarray(wo, np.float32),
    )
    res = run_on_hw(in_maps, trace=False)
    return _assemble(res.results)
